# revision 6
# baseline (speedup 1.0000x reference)
"""Trainium2 Bass kernel for the Actor MLP scorer (gnn_message_passing).

Computation (see reference):
    node_e  = node_embeddings[action_nodes]          # [A, 128] gather
    feats   = [node_e | region_embeddings[action_regions] | const_tail]   # [A, 1427]
    h1..h3  = relu MLP (256 wide), logits = h3 @ W4 + b4                  # [A]
    probs   = softmax(logits) over ALL actions

Strategy (8 NeuronCores, data-parallel over actions):
  - Shard A=100000 actions as 12500/core (padded to 12544 = 98*128).
  - Layer 1 is decomposed: feats @ W1 = node_e @ W1[:128]
        + onehot(region) @ (region_embeddings @ W1[128:256])
        + tail @ W1[256:]  (constant per action -> folded into bias).
    The constant tail/region projections are computed on-device in a prologue.
  - Activations are kept transposed ([feature, action]) so every layer's
    output feeds the next layer's matmul directly; the only transpose is the
    gathered node embeddings (PE transpose-mode).
  - Matmuls run as float32r (full-rate fp32 PE mode); PSUM accumulates fp32.
  - Softmax: per-core sum(exp(logit - 4)), one [1,1] AllReduce over the 8
    cores, then probs = exp * (1/S) on-core.  (Logit range here is ~+-3 so
    no max-subtraction is needed for fp32 safety; -4 adds margin.)
"""

import sys

for _p in ("/opt/trn_rl_repo",):
    if _p not in sys.path:
        sys.path.insert(0, _p)

import numpy as np
from concourse import bass, bacc, mybir, tile
from concourse import bass_utils
from concourse.masks import make_identity

# ---------------------------------------------------------------- constants
N_CORES = 8
A_FULL = 100000
N_NODES = 50000
N_REGIONS = 8
D = 128
H = 256
G = 147
IN_DIM = 2 * D + N_REGIONS * D + G          # 1427
TAIL_LEN = N_REGIONS * D + G                # 1171
TAIL_KT = 10                                # ceil(1171/128)
F32 = mybir.dt.float32
F32R = mybir.dt.float32r
I32 = mybir.dt.int32

A_PC = A_FULL // N_CORES                    # 12500
A_PAD = 12544                               # 98 * 128
N_CHUNKS = A_PAD // 128                     # 98
ATILE = 512

NEG_FILL = -1.0e9
EXP_SHIFT = -4.0


def _r(ap):
    """View an fp32 AP as float32r for full-rate PE matmuls."""
    return ap.bitcast(F32R)


def build_graph(a_pad=A_PAD, atile=ATILE):
    """Build the SPMD Bass graph for one core (all 8 run it)."""
    assert a_pad % 128 == 0
    n_chunks = a_pad // 128
    nc = bacc.Bacc("TRN2", target_bir_lowering=False, debug=False,
                   num_devices=N_CORES)

    # ---- I/O --------------------------------------------------------------
    node_emb = nc.dram_tensor("node_emb", [N_NODES, D], F32, kind="ExternalInput")
    w1 = nc.dram_tensor("w1", [IN_DIM, H], F32, kind="ExternalInput")
    w2 = nc.dram_tensor("w2", [H, H], F32, kind="ExternalInput")
    w3 = nc.dram_tensor("w3", [H, H], F32, kind="ExternalInput")
    w4c = nc.dram_tensor("w4c", [D, 2], F32, kind="ExternalInput")
    b1c_in = nc.dram_tensor("b1c", [D, 2], F32, kind="ExternalInput")
    b2c_in = nc.dram_tensor("b2c", [D, 2], F32, kind="ExternalInput")
    b3c_in = nc.dram_tensor("b3c", [D, 2], F32, kind="ExternalInput")
    b4_in = nc.dram_tensor("b4", [1, 1], F32, kind="ExternalInput")
    regT = nc.dram_tensor("regT", [D, N_REGIONS], F32, kind="ExternalInput")
    tailc = nc.dram_tensor("tailc", [D, TAIL_KT], F32, kind="ExternalInput")
    idxc = nc.dram_tensor("idxc", [128, n_chunks], I32, kind="ExternalInput")
    onehot = nc.dram_tensor("onehot", [N_REGIONS, a_pad], F32, kind="ExternalInput")

    out_logits = nc.dram_tensor("out_logits", [1, a_pad], F32, kind="ExternalOutput")
    out_probs = nc.dram_tensor("out_probs", [128, n_chunks], F32, kind="ExternalOutput")

    n_real = A_PC  # valid actions per core; rest are padding

    with tile.TileContext(nc) as tc:
        with (
            tc.tile_pool(name="const", bufs=1) as cpool,
            tc.tile_pool(name="work", bufs=3) as wpool,
            tc.tile_pool(name="hbuf", bufs=3) as hpool,
            tc.tile_pool(name="pnt", bufs=2, space="PSUM") as pnt_pool,
            tc.tile_pool(name="ph", bufs=3, space="PSUM") as ph_pool,
            tc.tile_pool(name="plg", bufs=1, space="PSUM") as plg_pool,
            tc.tile_pool(name="pmisc", bufs=2, space="PSUM") as pmisc_pool,
            tc.tile_pool(name="dram", bufs=1, space="DRAM") as dpool,
        ):
            # ---- constant loads ------------------------------------------
            ident = cpool.tile([128, 128], F32, tag="ident")
            make_identity(nc, ident[:])

            w1a = cpool.tile([128, H], F32R, tag="w1a")
            nc.sync.dma_start(out=w1a[:], in_=w1[0:D, :].bitcast(F32R))
            w1b = cpool.tile([128, H], F32, tag="w1b")
            nc.sync.dma_start(out=w1b[:], in_=w1[D:2 * D, :])
            w2t = [cpool.tile([128, H], F32R, tag=f"w2_{k}", name=f"w2_{k}") for k in range(2)]
            w3t = [cpool.tile([128, H], F32R, tag=f"w3_{k}", name=f"w3_{k}") for k in range(2)]
            for k in range(2):
                nc.sync.dma_start(out=w2t[k][:], in_=w2[k * 128:(k + 1) * 128, :].bitcast(F32R))
                nc.sync.dma_start(out=w3t[k][:], in_=w3[k * 128:(k + 1) * 128, :].bitcast(F32R))
            w4s = cpool.tile([128, 2], F32R, tag="w4s")
            nc.sync.dma_start(out=w4s[:], in_=w4c[:].bitcast(F32R))
            b2s = cpool.tile([128, 2], F32, tag="b2s")
            nc.sync.dma_start(out=b2s[:], in_=b2c_in[:])
            b3s = cpool.tile([128, 2], F32, tag="b3s")
            nc.sync.dma_start(out=b3s[:], in_=b3c_in[:])
            b4s = cpool.tile([1, 1], F32, tag="b4s")
            nc.sync.dma_start(out=b4s[:], in_=b4_in[:])
            b1s = cpool.tile([128, 2], F32, tag="b1s")
            nc.sync.dma_start(out=b1s[:], in_=b1c_in[:])
            regTs = cpool.tile([128, N_REGIONS], F32, tag="regTs")
            nc.sync.dma_start(out=regTs[:], in_=regT[:])
            tails = cpool.tile([128, TAIL_KT], F32, tag="tails")
            nc.sync.dma_start(out=tails[:], in_=tailc[:])
            idxs = cpool.tile([128, n_chunks], I32, tag="idxs")
            nc.sync.dma_start(out=idxs[:], in_=idxc[:])
            ohs = cpool.tile([N_REGIONS, a_pad], F32R, tag="ohs")
            nc.sync.dma_start(out=ohs[:], in_=onehot[:].bitcast(F32R))

            # ---- prologue: RP = region_emb @ W1b  (in [region, j] layout) -
            rp_ps = pmisc_pool.tile([8, H], F32, space="PSUM", tag="pm")
            nc.tensor.matmul(out=rp_ps[:], lhsT=regTs[:], rhs=w1b[:],
                             start=True, stop=True)
            rps = cpool.tile([8, H], F32R, tag="rps")
            nc.vector.tensor_copy(out=rps[:], in_=rp_ps[:])

            # ---- prologue: c_tail = tail @ W1[256:] + b1 ------------------
            w1tt = [cpool.tile([128, H], F32, tag=f"w1t_{kt}", name=f"w1t_{kt}") for kt in range(TAIL_KT)]
            for kt in range(TAIL_KT):
                r0 = 2 * D + kt * 128
                r1 = min(2 * D + (kt + 1) * 128, IN_DIM)
                nc.sync.dma_start(out=w1tt[kt][0:r1 - r0, :], in_=w1[r0:r1, :])
            ct_ps = pmisc_pool.tile([128, 2], F32, space="PSUM", tag="pm")
            for j in range(2):
                for kt in range(TAIL_KT):
                    kk = min(128, TAIL_LEN - kt * 128)
                    nc.tensor.matmul(
                        out=ct_ps[:, j:j + 1],
                        lhsT=w1tt[kt][0:kk, j * 128:(j + 1) * 128],
                        rhs=tails[0:kk, kt:kt + 1],
                        start=(kt == 0), stop=(kt == TAIL_KT - 1))
            b1cs = cpool.tile([128, 2], F32, tag="b1cs")
            nc.vector.tensor_add(out=b1cs[:], in0=ct_ps[:], in1=b1s[:])

            # logits accumulate here as one [1, a_pad] row
            lrow = cpool.tile([1, a_pad], F32, tag="lrow")

            # ---- main loop over action tiles ------------------------------
            n_at = (a_pad + atile - 1) // atile
            for t in range(n_at):
                base = t * atile
                S = min(atile, a_pad - base)
                nsub = S // 128

                graw = wpool.tile([128, atile], F32, tag="graw")
                for c in range(nsub):
                    cc = base // 128 + c
                    nc.gpsimd.indirect_dma_start(
                        out=graw[:, c * 128:(c + 1) * 128],
                        out_offset=None,
                        in_=node_emb[:],
                        in_offset=bass.IndirectOffsetOnAxis(
                            ap=idxs[:, cc:cc + 1], axis=0),
                    )
                # transpose gathered rows -> [dim, action]
                nt_ps = pnt_pool.tile([128, atile], F32, space="PSUM", tag="nt_ps")
                for c in range(nsub):
                    sl = slice(c * 128, (c + 1) * 128)
                    nc.tensor.transpose(out=nt_ps[:, sl], in_=graw[:, sl],
                                        identity=ident[:])
                nts = wpool.tile([128, atile], F32R, tag="nts")
                nc.vector.tensor_copy(out=nts[:, 0:S], in_=nt_ps[:, 0:S])

                # ---- layer 1: W1a.T @ node + RP.T @ onehot + b1c, relu ----
                h1 = [hpool.tile([128, atile], F32R, tag=f"h1_{j}", name=f"h1_{j}") for j in range(2)]
                for j in range(2):
                    hp = ph_pool.tile([128, atile], F32, space="PSUM", tag="hps")
                    nc.tensor.matmul(out=hp[:, 0:S],
                                     lhsT=w1a[:, j * 128:(j + 1) * 128],
                                     rhs=nts[:, 0:S], start=True, stop=False)
                    nc.tensor.matmul(out=hp[:, 0:S],
                                     lhsT=rps[0:8, j * 128:(j + 1) * 128],
                                     rhs=ohs[0:8, base:base + S],
                                     start=False, stop=True)
                    if j == 0:
                        nc.scalar.activation(
                            out=h1[j][:, 0:S], in_=hp[:, 0:S],
                            func=mybir.ActivationFunctionType.Relu,
                            bias=b1cs[:, j:j + 1])
                    else:
                        nc.vector.tensor_scalar(
                            out=h1[j][:, 0:S], in0=hp[:, 0:S],
                            scalar1=b1cs[:, j:j + 1], scalar2=0.0,
                            op0=mybir.AluOpType.add, op1=mybir.AluOpType.max)

                # ---- layers 2 and 3 --------------------------------------
                hin = h1
                for li, (wt, bs) in enumerate(((w2t, b2s), (w3t, b3s))):
                    hout = [hpool.tile([128, atile], F32R, tag=f"h{li + 2}_{j}",
                                      name=f"h{li + 2}_{j}")
                            for j in range(2)]
                    for j in range(2):
                        hp = ph_pool.tile([128, atile], F32, space="PSUM", tag="hps")
                        for k in range(2):
                            nc.tensor.matmul(
                                out=hp[:, 0:S],
                                lhsT=wt[k][:, j * 128:(j + 1) * 128],
                                rhs=hin[k][:, 0:S],
                                start=(k == 0), stop=(k == 1))
                        if (li + j) % 2 == 0:
                            nc.scalar.activation(
                                out=hout[j][:, 0:S], in_=hp[:, 0:S],
                                func=mybir.ActivationFunctionType.Relu,
                                bias=bs[:, j:j + 1])
                        else:
                            nc.vector.tensor_scalar(
                                out=hout[j][:, 0:S], in0=hp[:, 0:S],
                                scalar1=bs[:, j:j + 1], scalar2=0.0,
                                op0=mybir.AluOpType.add, op1=mybir.AluOpType.max)
                    hin = hout

                # ---- layer 4: logits = h3 @ W4 ----------------------------
                lg = plg_pool.tile([1, atile], F32, space="PSUM", tag="lg")
                for k in range(2):
                    nc.tensor.matmul(out=lg[:, 0:S], lhsT=w4s[:, k:k + 1],
                                     rhs=hin[k][:, 0:S],
                                     start=(k == 0), stop=(k == 1))
                nc.vector.tensor_scalar_add(
                    out=lrow[0:1, base:base + S], in0=lg[:, 0:S],
                    scalar1=b4s[0:1, 0:1])

            # ---- store logits ------------------------------------------
            nc.sync.dma_start(out=out_logits[:], in_=lrow[:])

            # ---- softmax ------------------------------------------------
            lgT = cpool.tile([128, n_chunks], F32, tag="lgT")
            nc.gpsimd.memset(lgT[:], NEG_FILL)
            full_p = n_real // n_chunks               # 127 full partitions
            rem = n_real - full_p * n_chunks          # leftover cols on last row
            nc.sync.dma_start(
                out=lgT[0:full_p, :],
                in_=out_logits[0:1, 0:full_p * n_chunks].rearrange(
                    "o (p t) -> (o p) t", p=full_p))
            if rem:
                nc.sync.dma_start(
                    out=lgT[full_p:full_p + 1, 0:rem],
                    in_=out_logits[0:1, full_p * n_chunks:n_real])

            expt = cpool.tile([128, n_chunks], F32, tag="expt")
            shift = cpool.tile([128, 1], F32, tag="shift")
            nc.gpsimd.memset(shift[:], EXP_SHIFT)
            nc.scalar.activation(out=expt[:], in_=lgT[:],
                                 func=mybir.ActivationFunctionType.Exp,
                                 bias=shift[:], scale=1.0)
            srow = cpool.tile([128, 1], F32, tag="srow")
            nc.vector.tensor_reduce(out=srow[:], in_=expt[:],
                                    axis=mybir.AxisListType.X,
                                    op=mybir.AluOpType.add)
            ones_col = cpool.tile([128, 1], F32, tag="ones_col")
            nc.gpsimd.memset(ones_col[:], 1.0)
            s_ps = pmisc_pool.tile([1, 1], F32, space="PSUM", tag="pm")
            nc.tensor.matmul(out=s_ps[:], lhsT=ones_col[:], rhs=srow[:],
                             start=True, stop=True)
            s_sb = cpool.tile([1, 1], F32, tag="s_sb")
            nc.vector.tensor_copy(out=s_sb[:], in_=s_ps[:])

            cc_in = dpool.tile([1, 1], F32, name="cc_in")
            cc_out = dpool.tile([1, 1], F32, addr_space="Shared", name="cc_out")
            nc.gpsimd.dma_start(out=cc_in[:], in_=s_sb[:])
            nc.gpsimd.collective_compute(
                "AllReduce", mybir.AluOpType.add,
                replica_groups=[list(range(N_CORES))],
                ins=[cc_in.opt()], outs=[cc_out.opt()])
            sg = cpool.tile([1, 1], F32, tag="sg")
            nc.gpsimd.dma_start(out=sg[:], in_=cc_out[:])

            rg = cpool.tile([1, 1], F32, tag="rg")
            nc.vector.reciprocal(out=rg[:], in_=sg[:])
            ones_row = cpool.tile([1, 128], F32, tag="ones_row")
            nc.gpsimd.memset(ones_row[:], 1.0)
            rb_ps = pmisc_pool.tile([128, 1], F32, space="PSUM", tag="pm")
            nc.tensor.matmul(out=rb_ps[:], lhsT=ones_row[:], rhs=rg[:],
                             start=True, stop=True)
            rb = cpool.tile([128, 1], F32, tag="rb")
            nc.vector.tensor_copy(out=rb[:], in_=rb_ps[:])

            probs = cpool.tile([128, n_chunks], F32, tag="probs")
            nc.vector.tensor_scalar_mul(out=probs[:], in0=expt[:], scalar1=rb[:])
            nc.sync.dma_start(out=out_probs[:], in_=probs[:])

    nc.compile()
    return nc


_GRAPH_CACHE = {}


def _get_graph():
    key = (A_PAD, ATILE)
    if key not in _GRAPH_CACHE:
        _GRAPH_CACHE[key] = build_graph()
    return _GRAPH_CACHE[key]


def make_in_maps(node_embeddings, region_embeddings, global_context,
                 W1, b1, W2, b2, W3, b3, W4, b4,
                 action_nodes, action_regions):
    """Host-side sharding / marshalling into per-core input dicts."""
    node_embeddings = np.ascontiguousarray(node_embeddings, dtype=np.float32)
    W1 = np.ascontiguousarray(W1, dtype=np.float32)
    W2 = np.ascontiguousarray(W2, dtype=np.float32)
    W3 = np.ascontiguousarray(W3, dtype=np.float32)
    an = np.asarray(action_nodes).astype(np.int32)
    ar = np.asarray(action_regions).astype(np.int32)

    tail = np.concatenate([
        np.asarray(region_embeddings, np.float32).reshape(-1),
        np.asarray(global_context, np.float32).reshape(-1)])
    tail_pad = np.zeros(TAIL_KT * 128, np.float32)
    tail_pad[:TAIL_LEN] = tail
    tailc = np.ascontiguousarray(tail_pad.reshape(TAIL_KT, 128).T)

    w4c = np.ascontiguousarray(np.asarray(W4, np.float32).reshape(2, 128).T)
    b1c = np.ascontiguousarray(np.asarray(b1, np.float32).reshape(2, 128).T)
    b2c = np.ascontiguousarray(np.asarray(b2, np.float32).reshape(2, 128).T)
    b3c = np.ascontiguousarray(np.asarray(b3, np.float32).reshape(2, 128).T)
    b4m = np.asarray(b4, np.float32).reshape(1, 1)
    regT = np.ascontiguousarray(np.asarray(region_embeddings, np.float32).T)

    in_maps = []
    for c in range(N_CORES):
        s = c * A_PC
        an_c = np.zeros(A_PAD, np.int32)
        an_c[:A_PC] = an[s:s + A_PC]
        ar_c = ar[s:s + A_PC]
        idxc = np.ascontiguousarray(an_c.reshape(N_CHUNKS, 128).T)
        oh = np.zeros((N_REGIONS, A_PAD), np.float32)
        oh[ar_c, np.arange(A_PC)] = 1.0
        in_maps.append({
            "node_emb": node_embeddings,
            "w1": W1, "w2": W2, "w3": W3,
            "w4c": w4c, "b1c": b1c, "b2c": b2c, "b3c": b3c, "b4": b4m,
            "regT": regT, "tailc": tailc,
            "idxc": idxc, "onehot": oh,
        })
    return in_maps


def kernel(**inputs):
    nc = _get_graph()
    in_maps = make_in_maps(**inputs)
    res = bass_utils.run_bass_kernel_spmd(
        nc, in_maps, core_ids=list(range(N_CORES)))
    probs = np.empty(A_FULL, np.float32)
    logits = np.empty(A_FULL, np.float32)
    for c in range(N_CORES):
        out = res.results[c]
        logits[c * A_PC:(c + 1) * A_PC] = out["out_logits"].reshape(-1)[:A_PC]
        probs[c * A_PC:(c + 1) * A_PC] = out["out_probs"].reshape(-1)[:A_PC]
    return probs, logits


# revision 8
# speedup vs baseline: 1.1916x; 1.1916x over previous
"""Trainium2 Bass kernel for the Actor MLP scorer (gnn_message_passing).

Computation (see reference):
    node_e  = node_embeddings[action_nodes]          # [A, 128] gather
    feats   = [node_e | region_embeddings[action_regions] | const_tail]   # [A, 1427]
    h1..h3  = relu MLP (256 wide), logits = h3 @ W4 + b4                  # [A]
    probs   = softmax(logits) over ALL actions

Strategy (8 NeuronCores, data-parallel over actions):
  - Shard A=100000 actions as 12500/core.  Per core, actions are sorted by
    node-id bucket (< 32768 vs >= 32768) so the node-embedding gather can use
    the int16-indexed DMA-gather ucode with two base-offset views of a bf16
    copy of the table; transpose mode deposits embeddings directly in
    [dim, action] layout (no on-chip transposes).  Groups are padded to the
    static capacities C0/C1 (~7 sigma for uniform node ids); a mask input
    removes pad slots from the softmax.  Outputs are un-permuted on host.
  - Layer 1 is decomposed: feats @ W1 = node_e @ W1[:128]
        + onehot(region) @ (region_embeddings @ W1[128:256])
        + tail @ W1[256:]  (constant -> folded into the relu bias).
    The constant tail/region projections are computed on-device in a
    fp32 prologue.
  - Activations stay transposed ([feature, action]); matmuls are bf16 with
    fp32 PSUM accumulation; relu+bias evictions split across ScalarE/VectorE.
  - Softmax: per-core sum(exp(logit - 4)), one [1,1] AllReduce over the 8
    cores, then probs = exp * (1/S) on-core.
"""

import sys

for _p in ("/opt/trn_rl_repo",):
    if _p not in sys.path:
        sys.path.insert(0, _p)

import numpy as np
import ml_dtypes
from concourse import bass, bacc, mybir, tile
from concourse import bass_utils

# ---------------------------------------------------------------- constants
N_CORES = 8
A_FULL = 100000
N_NODES = 50000
N_REGIONS = 8
D = 128
H = 256
G = 147
IN_DIM = 2 * D + N_REGIONS * D + G          # 1427
TAIL_LEN = N_REGIONS * D + G                # 1171
TAIL_KT = 10                                # ceil(1171/128)
F32 = mybir.dt.float32
BF16 = mybir.dt.bfloat16
I16 = mybir.dt.int16

A_PC = A_FULL // N_CORES                    # 12500
SPLIT = 32768                               # int16 index range boundary
C0 = 8576                                   # capacity, node id < 32768 (67*128)
C1 = 4736                                   # capacity, node id >= 32768 (37*128)
A_PAD = C0 + C1                             # 13312 = 26*512 = 104*128
N_CHUNKS = A_PAD // 128                     # 104
ATILE = 512
N_AT = A_PAD // ATILE                       # 26
GCHUNK = 512                                # idxs per dma_gather call

EXP_SHIFT = -4.0


def _gather_chunks(total):
    out, off = [], 0
    while off < total:
        n = min(GCHUNK, total - off)
        out.append((off, n))
        off += n
    return out


def build_graph():
    nc = bacc.Bacc("TRN2", target_bir_lowering=False, debug=False,
                   num_devices=N_CORES)

    # ---- I/O --------------------------------------------------------------
    node_emb = nc.dram_tensor("node_emb", [N_NODES, D], BF16, kind="ExternalInput")
    w1 = nc.dram_tensor("w1", [IN_DIM, H], F32, kind="ExternalInput")
    w2 = nc.dram_tensor("w2", [H, H], F32, kind="ExternalInput")
    w3 = nc.dram_tensor("w3", [H, H], F32, kind="ExternalInput")
    w4c = nc.dram_tensor("w4c", [D, 2], F32, kind="ExternalInput")
    b1c_in = nc.dram_tensor("b1c", [D, 2], F32, kind="ExternalInput")
    b2c_in = nc.dram_tensor("b2c", [D, 2], F32, kind="ExternalInput")
    b3c_in = nc.dram_tensor("b3c", [D, 2], F32, kind="ExternalInput")
    b4_in = nc.dram_tensor("b4", [1, 1], F32, kind="ExternalInput")
    regT = nc.dram_tensor("regT", [D, N_REGIONS], F32, kind="ExternalInput")
    tailc = nc.dram_tensor("tailc", [D, TAIL_KT], F32, kind="ExternalInput")
    idx0 = nc.dram_tensor("idx0", [128, C0 // 16], I16, kind="ExternalInput")
    idx1 = nc.dram_tensor("idx1", [128, C1 // 16], I16, kind="ExternalInput")
    onehot = nc.dram_tensor("onehot", [N_REGIONS, A_PAD], BF16, kind="ExternalInput")
    mask_in = nc.dram_tensor("mask", [128, N_CHUNKS], F32, kind="ExternalInput")

    out_logits = nc.dram_tensor("out_logits", [1, A_PAD], F32, kind="ExternalOutput")
    out_probs = nc.dram_tensor("out_probs", [128, N_CHUNKS], F32, kind="ExternalOutput")

    with tile.TileContext(nc) as tc:
        with (
            tc.tile_pool(name="const", bufs=1) as cpool,
            tc.tile_pool(name="hbuf", bufs=2) as hpool,
            tc.tile_pool(name="ph", bufs=4, space="PSUM") as ph_pool,
            tc.tile_pool(name="plg", bufs=2, space="PSUM") as plg_pool,
            tc.tile_pool(name="pmisc", bufs=2, space="PSUM") as pmisc_pool,
            tc.tile_pool(name="dram", bufs=1, space="DRAM") as dpool,
        ):
            # ---- constant loads (bf16 weights via SWDGE cast-DMA) --------
            w1a = cpool.tile([128, H], BF16, tag="w1a")
            nc.gpsimd.dma_start(out=w1a[:], in_=w1[0:D, :])
            w2t = [cpool.tile([128, H], BF16, tag=f"w2_{k}", name=f"w2_{k}")
                   for k in range(2)]
            w3t = [cpool.tile([128, H], BF16, tag=f"w3_{k}", name=f"w3_{k}")
                   for k in range(2)]
            for k in range(2):
                nc.gpsimd.dma_start(out=w2t[k][:], in_=w2[k * 128:(k + 1) * 128, :])
                nc.gpsimd.dma_start(out=w3t[k][:], in_=w3[k * 128:(k + 1) * 128, :])
            w4s = cpool.tile([128, 2], BF16, tag="w4s")
            nc.gpsimd.dma_start(out=w4s[:], in_=w4c[:])

            w1b = cpool.tile([128, H], F32, tag="w1b")
            nc.sync.dma_start(out=w1b[:], in_=w1[D:2 * D, :])
            b2s = cpool.tile([128, 2], F32, tag="b2s")
            nc.sync.dma_start(out=b2s[:], in_=b2c_in[:])
            b3s = cpool.tile([128, 2], F32, tag="b3s")
            nc.sync.dma_start(out=b3s[:], in_=b3c_in[:])
            b4s = cpool.tile([1, 1], F32, tag="b4s")
            nc.sync.dma_start(out=b4s[:], in_=b4_in[:])
            b1s = cpool.tile([128, 2], F32, tag="b1s")
            nc.sync.dma_start(out=b1s[:], in_=b1c_in[:])
            regTs = cpool.tile([128, N_REGIONS], F32, tag="regTs")
            nc.sync.dma_start(out=regTs[:], in_=regT[:])
            tails = cpool.tile([128, TAIL_KT], F32, tag="tails")
            nc.sync.dma_start(out=tails[:], in_=tailc[:])
            ohs = cpool.tile([N_REGIONS, A_PAD], BF16, tag="ohs")
            nc.sync.dma_start(out=ohs[:], in_=onehot[:])
            masks = cpool.tile([128, N_CHUNKS], F32, tag="masks")
            nc.sync.dma_start(out=masks[:], in_=mask_in[:])
            i0 = cpool.tile([128, C0 // 16], I16, tag="i0")
            nc.sync.dma_start(out=i0[:], in_=idx0[:])
            i1 = cpool.tile([128, C1 // 16], I16, tag="i1")
            nc.sync.dma_start(out=i1[:], in_=idx1[:])

            # ---- node gather: int16 dma_gather, transpose mode -----------
            # nts_all[d, slot] = node_emb[node_id(slot), d]  (bf16)
            nts_all = cpool.tile([128, A_PAD], BF16, tag="nts_all")
            for off, n in _gather_chunks(C0):
                nc.gpsimd.dma_gather(
                    out_ap=nts_all[:, off:off + n].rearrange(
                        "p (o n) -> p o n", o=1),
                    in_ap=node_emb[0:SPLIT, :],
                    idxs_ap=i0[:, off // 16:(off + n) // 16],
                    num_idxs=n, num_idxs_reg=n,
                    elem_size=D, transpose=True, single_packet=False)
            for off, n in _gather_chunks(C1):
                nc.gpsimd.dma_gather(
                    out_ap=nts_all[:, C0 + off:C0 + off + n].rearrange(
                        "p (o n) -> p o n", o=1),
                    in_ap=node_emb[SPLIT:N_NODES, :],
                    idxs_ap=i1[:, off // 16:(off + n) // 16],
                    num_idxs=n, num_idxs_reg=n,
                    elem_size=D, transpose=True, single_packet=False)

            # ---- prologue: RP = region_emb @ W1b (fp32, [region, j]) -----
            rp_ps = pmisc_pool.tile([8, H], F32, space="PSUM", tag="pm")
            nc.tensor.matmul(out=rp_ps[:], lhsT=regTs[:], rhs=w1b[:],
                             start=True, stop=True)
            rps = cpool.tile([8, H], BF16, tag="rps")
            nc.vector.tensor_copy(out=rps[:], in_=rp_ps[:])

            # ---- prologue: c_tail = tail @ W1[256:] + b1 (fp32) ----------
            w1tt = [cpool.tile([128, H], F32, tag=f"w1t_{kt}", name=f"w1t_{kt}")
                    for kt in range(TAIL_KT)]
            for kt in range(TAIL_KT):
                r0 = 2 * D + kt * 128
                r1 = min(2 * D + (kt + 1) * 128, IN_DIM)
                nc.sync.dma_start(out=w1tt[kt][0:r1 - r0, :], in_=w1[r0:r1, :])
            ct_ps = pmisc_pool.tile([128, 2], F32, space="PSUM", tag="pm")
            for j in range(2):
                for kt in range(TAIL_KT):
                    kk = min(128, TAIL_LEN - kt * 128)
                    nc.tensor.matmul(
                        out=ct_ps[:, j:j + 1],
                        lhsT=w1tt[kt][0:kk, j * 128:(j + 1) * 128],
                        rhs=tails[0:kk, kt:kt + 1],
                        start=(kt == 0), stop=(kt == TAIL_KT - 1))
            b1cs = cpool.tile([128, 2], F32, tag="b1cs")
            nc.vector.tensor_add(out=b1cs[:], in0=ct_ps[:], in1=b1s[:])

            lrow = cpool.tile([1, A_PAD], F32, tag="lrow")

            def evict_relu(engine, dst, src, bias_ap):
                if engine == "act":
                    nc.scalar.activation(
                        out=dst, in_=src,
                        func=mybir.ActivationFunctionType.Relu, bias=bias_ap)
                else:
                    nc.vector.tensor_scalar(
                        out=dst, in0=src, scalar1=bias_ap, scalar2=0.0,
                        op0=mybir.AluOpType.add, op1=mybir.AluOpType.max)

            # ---- main loop over action tiles ------------------------------
            for t in range(N_AT):
                base = t * ATILE
                sl = slice(base, base + ATILE)

                # layer 1
                h1 = [hpool.tile([128, ATILE], BF16, tag=f"h1_{j}", name=f"h1_{j}")
                      for j in range(2)]
                for j in range(2):
                    hp = ph_pool.tile([128, ATILE], F32, space="PSUM", tag="hps")
                    nc.tensor.matmul(out=hp[:],
                                     lhsT=w1a[:, j * 128:(j + 1) * 128],
                                     rhs=nts_all[:, sl], start=True, stop=False)
                    nc.tensor.matmul(out=hp[:],
                                     lhsT=rps[0:8, j * 128:(j + 1) * 128],
                                     rhs=ohs[0:8, sl], start=False, stop=True)
                    evict_relu("act" if j == 0 else "dve",
                               h1[j][:], hp[:], b1cs[:, j:j + 1])

                # layers 2 and 3
                hin = h1
                for li, (wt, bs) in enumerate(((w2t, b2s), (w3t, b3s))):
                    hout = [hpool.tile([128, ATILE], BF16, tag=f"h{li + 2}_{j}",
                                       name=f"h{li + 2}_{j}")
                            for j in range(2)]
                    for j in range(2):
                        hp = ph_pool.tile([128, ATILE], F32, space="PSUM",
                                          tag="hps")
                        for k in range(2):
                            nc.tensor.matmul(
                                out=hp[:],
                                lhsT=wt[k][:, j * 128:(j + 1) * 128],
                                rhs=hin[k][:],
                                start=(k == 0), stop=(k == 1))
                        # balance evictions: ScalarE gets j==0 (+ j==1 of L2 on
                        # odd tiles), VectorE the rest
                        eng = "act" if (j == 0 or (li == 0 and t % 2 == 1)) \
                            else "dve"
                        evict_relu(eng, hout[j][:], hp[:], bs[:, j:j + 1])
                    hin = hout

                # layer 4: logits
                lg = plg_pool.tile([1, ATILE], F32, space="PSUM", tag="lg")
                for k in range(2):
                    nc.tensor.matmul(out=lg[:], lhsT=w4s[:, k:k + 1],
                                     rhs=hin[k][:],
                                     start=(k == 0), stop=(k == 1))
                nc.vector.tensor_scalar_add(
                    out=lrow[0:1, sl], in0=lg[:], scalar1=b4s[0:1, 0:1])

            # ---- store logits -------------------------------------------
            nc.sync.dma_start(out=out_logits[:], in_=lrow[:])

            # ---- softmax ------------------------------------------------
            lgT = cpool.tile([128, N_CHUNKS], F32, tag="lgT")
            nc.sync.dma_start(
                out=lgT[:],
                in_=out_logits[0:1, :].rearrange("o (p t) -> (o p) t", p=128))
            expt = cpool.tile([128, N_CHUNKS], F32, tag="expt")
            shift = cpool.tile([128, 1], F32, tag="shift")
            nc.gpsimd.memset(shift[:], EXP_SHIFT)
            nc.scalar.activation(out=expt[:], in_=lgT[:],
                                 func=mybir.ActivationFunctionType.Exp,
                                 bias=shift[:], scale=1.0)
            em = cpool.tile([128, N_CHUNKS], F32, tag="em")
            nc.vector.tensor_tensor(out=em[:], in0=expt[:], in1=masks[:],
                                    op=mybir.AluOpType.mult)
            srow = cpool.tile([128, 1], F32, tag="srow")
            nc.vector.tensor_reduce(out=srow[:], in_=em[:],
                                    axis=mybir.AxisListType.X,
                                    op=mybir.AluOpType.add)
            ones_col = cpool.tile([128, 1], F32, tag="ones_col")
            nc.gpsimd.memset(ones_col[:], 1.0)
            s_ps = pmisc_pool.tile([1, 1], F32, space="PSUM", tag="pm")
            nc.tensor.matmul(out=s_ps[:], lhsT=ones_col[:], rhs=srow[:],
                             start=True, stop=True)
            s_sb = cpool.tile([1, 1], F32, tag="s_sb")
            nc.vector.tensor_copy(out=s_sb[:], in_=s_ps[:])

            cc_in = dpool.tile([1, 1], F32, name="cc_in")
            cc_out = dpool.tile([1, 1], F32, addr_space="Shared", name="cc_out")
            nc.gpsimd.dma_start(out=cc_in[:], in_=s_sb[:])
            nc.gpsimd.collective_compute(
                "AllReduce", mybir.AluOpType.add,
                replica_groups=[list(range(N_CORES))],
                ins=[cc_in.opt()], outs=[cc_out.opt()])
            sg = cpool.tile([1, 1], F32, tag="sg")
            nc.gpsimd.dma_start(out=sg[:], in_=cc_out[:])

            rg = cpool.tile([1, 1], F32, tag="rg")
            nc.vector.reciprocal(out=rg[:], in_=sg[:])
            ones_row = cpool.tile([1, 128], F32, tag="ones_row")
            nc.gpsimd.memset(ones_row[:], 1.0)
            rb_ps = pmisc_pool.tile([128, 1], F32, space="PSUM", tag="pm")
            nc.tensor.matmul(out=rb_ps[:], lhsT=ones_row[:], rhs=rg[:],
                             start=True, stop=True)
            rb = cpool.tile([128, 1], F32, tag="rb")
            nc.vector.tensor_copy(out=rb[:], in_=rb_ps[:])

            probs = cpool.tile([128, N_CHUNKS], F32, tag="probs")
            nc.vector.tensor_scalar_mul(out=probs[:], in0=em[:], scalar1=rb[:])
            nc.sync.dma_start(out=out_probs[:], in_=probs[:])

    nc.compile()
    return nc


_GRAPH_CACHE = {}


def _get_graph():
    if "g" not in _GRAPH_CACHE:
        _GRAPH_CACHE["g"] = build_graph()
    return _GRAPH_CACHE["g"]


def _wrap_idx(ix):
    """int16 index layout for dma_gather: [16, N/16] column-wrapped,
    replicated 8x down the partitions."""
    w = ix.reshape(-1, 16).T
    return np.ascontiguousarray(np.tile(w, (8, 1)))


def make_in_maps(node_embeddings, region_embeddings, global_context,
                 W1, b1, W2, b2, W3, b3, W4, b4,
                 action_nodes, action_regions):
    """Host-side sharding / marshalling. Returns (in_maps, per-core metas)."""
    W1 = np.ascontiguousarray(W1, dtype=np.float32)
    W2 = np.ascontiguousarray(W2, dtype=np.float32)
    W3 = np.ascontiguousarray(W3, dtype=np.float32)
    an = np.asarray(action_nodes).astype(np.int64)
    ar = np.asarray(action_regions).astype(np.int64)
    node_bf16 = np.ascontiguousarray(
        np.asarray(node_embeddings, np.float32).astype(ml_dtypes.bfloat16))

    tail = np.concatenate([
        np.asarray(region_embeddings, np.float32).reshape(-1),
        np.asarray(global_context, np.float32).reshape(-1)])
    tail_pad = np.zeros(TAIL_KT * 128, np.float32)
    tail_pad[:TAIL_LEN] = tail
    tailc = np.ascontiguousarray(tail_pad.reshape(TAIL_KT, 128).T)

    w4c = np.ascontiguousarray(np.asarray(W4, np.float32).reshape(2, 128).T)
    b1c = np.ascontiguousarray(np.asarray(b1, np.float32).reshape(2, 128).T)
    b2c = np.ascontiguousarray(np.asarray(b2, np.float32).reshape(2, 128).T)
    b3c = np.ascontiguousarray(np.asarray(b3, np.float32).reshape(2, 128).T)
    b4m = np.asarray(b4, np.float32).reshape(1, 1)
    regTm = np.ascontiguousarray(np.asarray(region_embeddings, np.float32).T)

    in_maps, metas = [], []
    for c in range(N_CORES):
        s = c * A_PC
        nodes = an[s:s + A_PC]
        regions = ar[s:s + A_PC]
        grp = (nodes >= SPLIT).astype(np.int8)
        order = np.argsort(grp, kind="stable")      # group0 first, stable
        c0 = int((grp == 0).sum())
        c1 = A_PC - c0
        if c0 > C0 or c1 > C1:
            raise RuntimeError(
                f"core {c}: group sizes {c0}/{c1} exceed capacities {C0}/{C1}")
        sn = nodes[order]
        sr = regions[order]

        ix0 = np.zeros(C0, np.int16)
        ix0[:c0] = sn[:c0].astype(np.int16)
        ix1 = np.zeros(C1, np.int16)
        ix1[:c1] = (sn[c0:] - SPLIT).astype(np.int16)

        slots = np.concatenate([np.arange(c0), C0 + np.arange(c1)])
        oh = np.zeros((N_REGIONS, A_PAD), ml_dtypes.bfloat16)
        oh[sr, slots] = 1.0
        mask = np.zeros(A_PAD, np.float32)
        mask[slots] = 1.0

        in_maps.append({
            "node_emb": node_bf16,
            "w1": W1, "w2": W2, "w3": W3,
            "w4c": w4c, "b1c": b1c, "b2c": b2c, "b3c": b3c, "b4": b4m,
            "regT": regTm, "tailc": tailc,
            "idx0": _wrap_idx(ix0), "idx1": _wrap_idx(ix1),
            "onehot": oh, "mask": mask.reshape(128, N_CHUNKS),
        })
        metas.append((order, slots))
    return in_maps, metas


def kernel(**inputs):
    nc = _get_graph()
    in_maps, metas = make_in_maps(**inputs)
    res = bass_utils.run_bass_kernel_spmd(
        nc, in_maps, core_ids=list(range(N_CORES)))
    probs = np.empty(A_FULL, np.float32)
    logits = np.empty(A_FULL, np.float32)
    for c in range(N_CORES):
        order, slots = metas[c]
        out = res.results[c]
        lg = out["out_logits"].reshape(-1)[slots]
        pb = out["out_probs"].reshape(-1)[slots]
        logits[c * A_PC:(c + 1) * A_PC][order] = lg
        probs[c * A_PC:(c + 1) * A_PC][order] = pb
    return probs, logits
